# revision 1
# baseline (speedup 1.0000x reference)
"""Deformable-Transformer encoder on 8 trn2 NeuronCores.

Sharding: data-parallel over batch x token-parallel within batch
(8 cores = 2 batches x 4 token-shards of 1360 tokens).

Device programs (bass/Tile, SPMD on cores 0-7):
  A: value/offset/attn projections + softmax(attn weights)
  B: attn output proj + residual + LayerNorm1
  C: FFN first matmul + ReLU
  D: FFN second matmul + residual + LayerNorm2
The data-dependent bilinear sampling (sparse gather; this terminal's
runtime cannot load the GPSIMD gather ucode libraries) runs on host
between launches. Host also reshards/transposes between launches.
"""
import os
import sys
import types
import contextlib
import ctypes
import numpy as np

sys.path.insert(0, "/opt/trn_rl_repo")


def _install_ntff_hook():
    try:
        import antenv

        if hasattr(antenv, "axon_hooks"):
            return
        so_path = "/opt/axon/libaxon_pjrt.so"
        lib = ctypes.CDLL(so_path)
        if not hasattr(lib, "axon_start_nrt_profile"):
            hook = None
        else:
            lib.axon_start_nrt_profile.argtypes = [
                ctypes.POINTER(ctypes.c_int64), ctypes.c_size_t]
            lib.axon_start_nrt_profile.restype = ctypes.c_int64
            lib.axon_stop_nrt_profile.argtypes = [ctypes.c_char_p]
            lib.axon_stop_nrt_profile.restype = ctypes.c_int64

            @contextlib.contextmanager
            def hook(output_dir, device_ids):
                import jax
                jax.devices()
                if device_ids:
                    ids = (ctypes.c_int64 * len(device_ids))(*device_ids)
                    rc = lib.axon_start_nrt_profile(ids, len(device_ids))
                else:
                    rc = lib.axon_start_nrt_profile(None, 0)
                if rc != 0:
                    raise RuntimeError(f"start_nrt_profile rc={rc}")
                try:
                    yield
                finally:
                    lib.axon_stop_nrt_profile(str(output_dir).encode())

        m = types.ModuleType("antenv.axon_hooks")
        m.get_axon_ntff_profile_hook = lambda: hook
        m.set_axon_ntff_profile_hook = lambda h: None
        sys.modules["antenv.axon_hooks"] = m
        antenv.axon_hooks = m
    except Exception:
        pass


_install_ntff_hook()

from concourse import bacc, tile, mybir, bass  # noqa: E402
from concourse.bass_utils import run_bass_kernel_spmd  # noqa: E402
from contextlib import ExitStack  # noqa: E402

F32 = mybir.dt.float32

SHAPES = ((64, 64), (32, 32), (16, 16), (8, 8))
LEVEL_STARTS = [0, 4096, 5120, 5376, 5440]
N_LEVELS, N_HEADS, N_POINTS = 4, 8, 4
D_MODEL, HEAD_DIM, D_FFN = 256, 32, 1024
LEN_IN, BATCH, NCORE = 5440, 2, 8
TPC = LEN_IN * BATCH // NCORE  # 1360 tokens per core

HW_EXEC_NS = []  # per-launch exec times when BASS_TRACE=1
_PROGS = {}


def _nc():
    return bacc.Bacc("TRN2", target_bir_lowering=False, debug=False,
                     num_devices=NCORE)


def _qtiles():
    out = []
    q0 = 0
    while q0 < TPC:
        out.append((q0, min(128, TPC - q0)))
        q0 += 128
    return out


def _ln(nc, sb, r_ap, g_t, be_t, out_t, sz, tag, eps_t, z_t):
    """out = LN(r) * g + be over free axis (256), r_ap [sz,256]."""
    m = sb.tile([128, 1], F32, tag=tag + "m")
    nc.vector.tensor_reduce(m[:sz], r_ap, axis=mybir.AxisListType.X,
                            op=mybir.AluOpType.add)
    mneg = sb.tile([128, 1], F32, tag=tag + "mn")
    nc.scalar.mul(mneg[:sz], m[:sz], -1.0 / D_MODEL)
    xc = sb.tile([128, D_MODEL], F32, tag=tag + "xc")
    nc.scalar.activation(xc[:sz], r_ap, mybir.ActivationFunctionType.Identity,
                         bias=mneg[:sz, :1])
    sq = sb.tile([128, D_MODEL], F32, tag=tag + "sq")
    nc.vector.tensor_tensor(sq[:sz], xc[:sz], xc[:sz],
                            op=mybir.AluOpType.mult)
    v = sb.tile([128, 1], F32, tag=tag + "v")
    nc.vector.tensor_reduce(v[:sz], sq[:sz], axis=mybir.AxisListType.X,
                            op=mybir.AluOpType.add)
    sd = sb.tile([128, 1], F32, tag=tag + "sd")
    # sd = sqrt(v/D + eps) via Sqrt(scale*v + bias)
    nc.scalar.activation(sd[:sz], v[:sz], mybir.ActivationFunctionType.Sqrt,
                         bias=eps_t[:sz, :1], scale=1.0 / D_MODEL)
    rs = sb.tile([128, 1], F32, tag=tag + "rs")
    nc.vector.reciprocal(rs[:sz], sd[:sz])
    xn = sb.tile([128, D_MODEL], F32, tag=tag + "xn")
    nc.scalar.activation(xn[:sz], xc[:sz],
                         mybir.ActivationFunctionType.Identity,
                         scale=rs[:sz, :1], bias=z_t[:sz, :1])
    nc.vector.tensor_tensor(xn[:sz], xn[:sz], g_t[:sz],
                            op=mybir.AluOpType.mult)
    nc.vector.tensor_tensor(out_t[:sz], xn[:sz], be_t[:sz],
                            op=mybir.AluOpType.add)


def _build_A():
    """in: xT[256,TPC], qT[256,TPC], Wv[256,256], Woa[256,384],
    bv_r[128,256], boa_r[128,384] -> val[TPC,256], offaw[TPC,384]."""
    nc = _nc()
    xT_d = nc.dram_tensor("xT", [D_MODEL, TPC], F32, kind="ExternalInput").ap()
    qT_d = nc.dram_tensor("qT", [D_MODEL, TPC], F32, kind="ExternalInput").ap()
    wv_d = nc.dram_tensor("Wv", [D_MODEL, 256], F32, kind="ExternalInput").ap()
    woa_d = nc.dram_tensor("Woa", [D_MODEL, 384], F32,
                           kind="ExternalInput").ap()
    bv_d = nc.dram_tensor("bv_r", [128, 256], F32, kind="ExternalInput").ap()
    boa_d = nc.dram_tensor("boa_r", [128, 384], F32, kind="ExternalInput").ap()
    val_d = nc.dram_tensor("val", [TPC, 256], F32, kind="ExternalOutput").ap()
    oa_d = nc.dram_tensor("offaw", [TPC, 384], F32, kind="ExternalOutput").ap()

    with tile.TileContext(nc) as tc, ExitStack() as ctx:
        sb = ctx.enter_context(tc.tile_pool(name="sb", bufs=1))
        ps = ctx.enter_context(tc.tile_pool(name="ps", bufs=4, space="PSUM"))
        ob = ctx.enter_context(tc.tile_pool(name="ob", bufs=3))

        xT = sb.tile([128, 2, TPC], F32, tag="xT")
        nc.sync.dma_start(xT[:], xT_d.rearrange("(c p) n -> p c n", p=128))
        qT = sb.tile([128, 2, TPC], F32, tag="qT")
        nc.sync.dma_start(qT[:], qT_d.rearrange("(c p) n -> p c n", p=128))
        wv = sb.tile([128, 2, 256], F32, tag="wv")
        nc.sync.dma_start(wv[:], wv_d.rearrange("(c p) n -> p c n", p=128))
        woa = sb.tile([128, 2, 384], F32, tag="woa")
        nc.sync.dma_start(woa[:], woa_d.rearrange("(c p) n -> p c n", p=128))
        bv = sb.tile([128, 256], F32, tag="bv")
        nc.sync.dma_start(bv[:], bv_d[:])
        boa = sb.tile([128, 384], F32, tag="boa")
        nc.sync.dma_start(boa[:], boa_d[:])

        for q0, sz in _qtiles():
            pv = ps.tile([128, 256], F32, tag="pv")
            for k in range(2):
                nc.tensor.matmul(pv[:sz], xT[:, k, q0:q0 + sz], wv[:, k, :],
                                 start=(k == 0), stop=(k == 1))
            ov = ob.tile([128, 256], F32, tag="ov")
            nc.vector.tensor_tensor(ov[:sz], pv[:sz], bv[:sz],
                                    op=mybir.AluOpType.add)
            nc.sync.dma_start(val_d[q0:q0 + sz, :], ov[:sz])

            po = ps.tile([128, 384], F32, tag="po")
            for k in range(2):
                nc.tensor.matmul(po[:sz], qT[:, k, q0:q0 + sz], woa[:, k, :],
                                 start=(k == 0), stop=(k == 1))
            oo = ob.tile([128, 384], F32, tag="oo")
            nc.vector.tensor_tensor(oo[:sz], po[:sz], boa[:sz],
                                    op=mybir.AluOpType.add)
            nc.sync.dma_start(oa_d[q0:q0 + sz, :], oo[:sz])
    nc.compile()
    return nc


def _build_B():
    """in: x[TPC,256], attnT[256,TPC], Wo, bo_r, g1_r, be1_r -> x2[TPC,256]"""
    nc = _nc()
    x_d = nc.dram_tensor("x", [TPC, 256], F32, kind="ExternalInput").ap()
    aT_d = nc.dram_tensor("attnT", [256, TPC], F32, kind="ExternalInput").ap()
    wo_d = nc.dram_tensor("Wo", [256, 256], F32, kind="ExternalInput").ap()
    bo_d = nc.dram_tensor("bo_r", [128, 256], F32, kind="ExternalInput").ap()
    g1_d = nc.dram_tensor("g1_r", [128, 256], F32, kind="ExternalInput").ap()
    be1_d = nc.dram_tensor("be1_r", [128, 256], F32, kind="ExternalInput").ap()
    x2_d = nc.dram_tensor("x2", [TPC, 256], F32, kind="ExternalOutput").ap()

    with tile.TileContext(nc) as tc, ExitStack() as ctx:
        sb = ctx.enter_context(tc.tile_pool(name="sb", bufs=1))
        ps = ctx.enter_context(tc.tile_pool(name="ps", bufs=4, space="PSUM"))
        ob = ctx.enter_context(tc.tile_pool(name="ob", bufs=3))

        aT = sb.tile([128, 2, TPC], F32, tag="aT")
        nc.sync.dma_start(aT[:], aT_d.rearrange("(c p) n -> p c n", p=128))
        wo = sb.tile([128, 2, 256], F32, tag="wo")
        nc.sync.dma_start(wo[:], wo_d.rearrange("(c p) n -> p c n", p=128))
        bo = sb.tile([128, 256], F32, tag="bo")
        nc.sync.dma_start(bo[:], bo_d[:])
        g1 = sb.tile([128, 256], F32, tag="g1")
        nc.sync.dma_start(g1[:], g1_d[:])
        be1 = sb.tile([128, 256], F32, tag="be1")
        nc.sync.dma_start(be1[:], be1_d[:])

        for q0, sz in _qtiles():
            xt = ob.tile([128, 256], F32, tag="xt")
            nc.sync.dma_start(xt[:sz], x_d[q0:q0 + sz, :])
            p = ps.tile([128, 256], F32, tag="p")
            for k in range(2):
                nc.tensor.matmul(p[:sz], aT[:, k, q0:q0 + sz], wo[:, k, :],
                                 start=(k == 0), stop=(k == 1))
            r = ob.tile([128, 256], F32, tag="r")
            nc.vector.tensor_tensor(r[:sz], p[:sz], bo[:sz],
                                    op=mybir.AluOpType.add)
            nc.vector.tensor_tensor(r[:sz], r[:sz], xt[:sz],
                                    op=mybir.AluOpType.add)
            nc.sync.dma_start(x2_d[q0:q0 + sz, :], r[:sz])
    nc.compile()
    return nc


def _build_C():
    """in: x2T[256,TPC], Wl1[256,1024], bl1_r[128,1024] -> h[TPC,1024]"""
    nc = _nc()
    xT_d = nc.dram_tensor("x2T", [256, TPC], F32, kind="ExternalInput").ap()
    w_d = nc.dram_tensor("Wl1", [256, 1024], F32, kind="ExternalInput").ap()
    b_d = nc.dram_tensor("bl1_r", [128, 1024], F32, kind="ExternalInput").ap()
    h_d = nc.dram_tensor("h", [TPC, 1024], F32, kind="ExternalOutput").ap()

    with tile.TileContext(nc) as tc, ExitStack() as ctx:
        sb = ctx.enter_context(tc.tile_pool(name="sb", bufs=1))
        ps = ctx.enter_context(tc.tile_pool(name="ps", bufs=4, space="PSUM"))
        ob = ctx.enter_context(tc.tile_pool(name="ob", bufs=3))

        xT = sb.tile([128, 2, TPC], F32, tag="xT")
        nc.sync.dma_start(xT[:], xT_d.rearrange("(c p) n -> p c n", p=128))
        w = sb.tile([128, 2, 1024], F32, tag="w")
        nc.sync.dma_start(w[:], w_d.rearrange("(c p) n -> p c n", p=128))
        b = sb.tile([128, 1024], F32, tag="b")
        nc.sync.dma_start(b[:], b_d[:])
        z512 = sb.tile([128, 512], F32, tag="z512")
        nc.gpsimd.memset(z512[:], 0.0)

        for q0, sz in _qtiles():
            for n0 in range(0, 1024, 512):
                p = ps.tile([128, 512], F32, tag="p")
                for k in range(2):
                    nc.tensor.matmul(p[:sz], xT[:, k, q0:q0 + sz],
                                     w[:, k, n0:n0 + 512],
                                     start=(k == 0), stop=(k == 1))
                t = ob.tile([128, 512], F32, tag="t")
                nc.vector.tensor_tensor(t[:sz], p[:sz], b[:sz, n0:n0 + 512],
                                        op=mybir.AluOpType.add)
                o = ob.tile([128, 512], F32, tag="o")
                nc.vector.tensor_tensor(o[:sz], t[:sz], z512[:sz],
                                        op=mybir.AluOpType.max)
                nc.sync.dma_start(h_d[q0:q0 + sz, n0:n0 + 512], o[:sz])
    nc.compile()
    return nc


def _build_D():
    """in: hT[1024,TPC], Wl2[1024,256], bl2_r, x2[TPC,256], g2_r, be2_r
    -> out[TPC,256]"""
    nc = _nc()
    hT_d = nc.dram_tensor("hT", [D_FFN, TPC], F32, kind="ExternalInput").ap()
    w_d = nc.dram_tensor("Wl2", [D_FFN, 256], F32, kind="ExternalInput").ap()
    b_d = nc.dram_tensor("bl2_r", [128, 256], F32, kind="ExternalInput").ap()
    x2_d = nc.dram_tensor("x2", [TPC, 256], F32, kind="ExternalInput").ap()
    g2_d = nc.dram_tensor("g2_r", [128, 256], F32, kind="ExternalInput").ap()
    be2_d = nc.dram_tensor("be2_r", [128, 256], F32,
                           kind="ExternalInput").ap()
    o_d = nc.dram_tensor("out", [TPC, 256], F32, kind="ExternalOutput").ap()

    with tile.TileContext(nc) as tc, ExitStack() as ctx:
        sb = ctx.enter_context(tc.tile_pool(name="sb", bufs=1))
        ps = ctx.enter_context(tc.tile_pool(name="ps", bufs=4, space="PSUM"))
        ob = ctx.enter_context(tc.tile_pool(name="ob", bufs=3))

        hT = sb.tile([128, 8, TPC], F32, tag="hT")
        nc.sync.dma_start(hT[:], hT_d.rearrange("(c p) n -> p c n", p=128))
        w = sb.tile([128, 8, 256], F32, tag="w")
        nc.sync.dma_start(w[:], w_d.rearrange("(c p) n -> p c n", p=128))
        b = sb.tile([128, 256], F32, tag="b")
        nc.sync.dma_start(b[:], b_d[:])
        g2 = sb.tile([128, 256], F32, tag="g2")
        nc.sync.dma_start(g2[:], g2_d[:])
        be2 = sb.tile([128, 256], F32, tag="be2")
        nc.sync.dma_start(be2[:], be2_d[:])

        for q0, sz in _qtiles():
            xt = ob.tile([128, 256], F32, tag="xt")
            nc.sync.dma_start(xt[:sz], x2_d[q0:q0 + sz, :])
            p = ps.tile([128, 256], F32, tag="p")
            for k in range(8):
                nc.tensor.matmul(p[:sz], hT[:, k, q0:q0 + sz], w[:, k, :],
                                 start=(k == 0), stop=(k == 7))
            r = ob.tile([128, 256], F32, tag="r")
            nc.vector.tensor_tensor(r[:sz], p[:sz], b[:sz],
                                    op=mybir.AluOpType.add)
            nc.vector.tensor_tensor(r[:sz], r[:sz], xt[:sz],
                                    op=mybir.AluOpType.add)
            nc.sync.dma_start(o_d[q0:q0 + sz, :], r[:sz])
    nc.compile()
    return nc


def _run(prog, in_maps):
    trace = bool(os.environ.get("BASS_TRACE"))
    res = run_bass_kernel_spmd(prog, in_maps, core_ids=list(range(NCORE)),
                               trace=trace)
    if res.exec_time_ns:
        HW_EXEC_NS.append(res.exec_time_ns)
    return res.results


def _rep(v):
    return np.ascontiguousarray(np.broadcast_to(v[None, :], (128, v.shape[0])),
                                dtype=np.float32)


def _ref_points(valid_ratios):
    refs = []
    for lvl, (H, W) in enumerate(SHAPES):
        gy, gx = np.meshgrid(np.arange(H, dtype=np.float32) + 0.5,
                             np.arange(W, dtype=np.float32) + 0.5,
                             indexing="ij")
        ry = gy.reshape(-1)[None] / (valid_ratios[:, lvl, 1][:, None] * H)
        rx = gx.reshape(-1)[None] / (valid_ratios[:, lvl, 0][:, None] * W)
        refs.append(np.stack([rx, ry], -1))
    ref = np.concatenate(refs, 1)
    return ref[:, :, None, :] * valid_ratios[:, None]


def _host_ln(x, g, b, eps=1e-5):
    m = x.mean(-1, keepdims=True)
    v = np.square(x - m).mean(-1, keepdims=True)
    return ((x - m) / np.sqrt(v + eps) * g + b).astype(np.float32)


def _host_sample(value, off, aw, ref_pts):
    """value[N,L,8,32] off[N,L,256] aw[N,L,128](softmaxed) -> [N,L,256]"""
    N, Lq = off.shape[:2]
    off = off.reshape(N, Lq, N_HEADS, N_LEVELS, N_POINTS, 2)
    aw = aw.reshape(N, Lq, N_HEADS, N_LEVELS, N_POINTS)
    normalizer = np.array([[w, h] for h, w in SHAPES], np.float32)
    loc = (ref_pts[:, :, None, :, None, :]
           + off / normalizer[None, None, None, :, None, :])
    acc = np.zeros((N, N_HEADS, Lq, HEAD_DIM), np.float32)
    for lvl, (H, W) in enumerate(SHAPES):
        s = LEVEL_STARTS[lvl]
        val = value[:, s:s + H * W].transpose(0, 2, 1, 3)
        x = loc[:, :, :, lvl, :, 0] * W - 0.5
        y = loc[:, :, :, lvl, :, 1] * H - 0.5
        x0 = np.floor(x)
        y0 = np.floor(y)
        wx1 = x - x0
        wy1 = y - y0
        ix0 = x0.astype(np.int64)
        iy0 = y0.astype(np.int64)

        def corner(ix, iy, w):
            valid = (ix >= 0) & (ix < W) & (iy >= 0) & (iy < H)
            idx = np.clip(iy, 0, H - 1) * W + np.clip(ix, 0, W - 1)
            idx = idx.transpose(0, 2, 1, 3).reshape(N, N_HEADS, Lq * N_POINTS)
            g = np.take_along_axis(val, idx[..., None], axis=2)
            g = g.reshape(N, N_HEADS, Lq, N_POINTS, HEAD_DIM)
            w = np.where(valid, w, 0.0).transpose(0, 2, 1, 3)
            return g * w[..., None].astype(np.float32)

        sampled = (corner(ix0, iy0, (1 - wx1) * (1 - wy1))
                   + corner(ix0 + 1, iy0, wx1 * (1 - wy1))
                   + corner(ix0, iy0 + 1, (1 - wx1) * wy1)
                   + corner(ix0 + 1, iy0 + 1, wx1 * wy1))
        acc += (sampled * aw[:, :, :, lvl].transpose(0, 2, 1, 3)[..., None]
                ).sum(3)
    return acc.transpose(0, 2, 1, 3).reshape(N, Lq, D_MODEL)


def kernel(src, pos, valid_ratios, Wv, bv, Woff, boff, Wa, ba, Wo, bo,
           g1, be1, Wl1, bl1, Wl2, bl2, g2, be2):
    src = np.asarray(src, np.float32)
    pos = np.asarray(pos, np.float32)
    valid_ratios = np.asarray(valid_ratios, np.float32)
    HW_EXEC_NS.clear()

    if "A" not in _PROGS:
        _PROGS["A"] = _build_A()
        _PROGS["B"] = _build_B()
        _PROGS["C"] = _build_C()
        _PROGS["D"] = _build_D()

    ref_pts = _ref_points(valid_ratios)

    def shard(full):  # [2,5440,F] -> list of 8 [TPC,F]
        return [np.ascontiguousarray(full[c // 4, (c % 4) * TPC:
                                          (c % 4 + 1) * TPC])
                for c in range(NCORE)]

    def unshard(parts):  # list of 8 [TPC,F] -> [2,5440,F]
        F = parts[0].shape[-1]
        out = np.empty((BATCH, LEN_IN, F), np.float32)
        for c in range(NCORE):
            out[c // 4, (c % 4) * TPC:(c % 4 + 1) * TPC] = parts[c]
        return out

    x = src.copy()
    for layer in range(2):
        Woa = np.ascontiguousarray(
            np.concatenate([np.asarray(Woff[layer]), np.asarray(Wa[layer])],
                           axis=1), dtype=np.float32)
        boa = np.concatenate([np.asarray(boff[layer]), np.asarray(ba[layer])])
        xs = shard(x)
        qs = shard(x + pos)
        in_maps = [{
            "xT": np.ascontiguousarray(xs[c].T),
            "qT": np.ascontiguousarray(qs[c].T),
            "Wv": np.asarray(Wv[layer], np.float32),
            "Woa": Woa,
            "bv_r": _rep(np.asarray(bv[layer], np.float32)),
            "boa_r": _rep(boa.astype(np.float32)),
        } for c in range(NCORE)]
        resA = _run(_PROGS["A"], in_maps)
        value = unshard([resA[c]["val"] for c in range(NCORE)])
        offaw = unshard([resA[c]["offaw"] for c in range(NCORE)])
        aw = offaw[:, :, 256:].reshape(BATCH, LEN_IN, N_HEADS, 16)
        aw = aw - aw.max(-1, keepdims=True)
        e = np.exp(aw)
        aw = (e / e.sum(-1, keepdims=True)).reshape(BATCH, LEN_IN, 128)

        attn = _host_sample(
            value.reshape(BATCH, LEN_IN, N_HEADS, HEAD_DIM),
            offaw[:, :, :256], aw, ref_pts)

        ats = shard(attn)
        in_maps = [{
            "x": xs[c],
            "attnT": np.ascontiguousarray(ats[c].T),
            "Wo": np.asarray(Wo[layer], np.float32),
            "bo_r": _rep(np.asarray(bo[layer], np.float32)),
            "g1_r": _rep(np.asarray(g1[layer], np.float32)),
            "be1_r": _rep(np.asarray(be1[layer], np.float32)),
        } for c in range(NCORE)]
        resB = _run(_PROGS["B"], in_maps)
        x2f = unshard([resB[c]["x2"] for c in range(NCORE)])
        x2f = _host_ln(x2f, np.asarray(g1[layer]), np.asarray(be1[layer]))
        x2s = shard(x2f)

        in_maps = [{
            "x2T": np.ascontiguousarray(x2s[c].T),
            "Wl1": np.asarray(Wl1[layer], np.float32),
            "bl1_r": _rep(np.asarray(bl1[layer], np.float32)),
        } for c in range(NCORE)]
        resC = _run(_PROGS["C"], in_maps)

        in_maps = [{
            "hT": np.ascontiguousarray(resC[c]["h"].T),
            "Wl2": np.asarray(Wl2[layer], np.float32),
            "bl2_r": _rep(np.asarray(bl2[layer], np.float32)),
            "x2": x2s[c],
            "g2_r": _rep(np.asarray(g2[layer], np.float32)),
            "be2_r": _rep(np.asarray(be2[layer], np.float32)),
        } for c in range(NCORE)]
        resD = _run(_PROGS["D"], in_maps)
        x = unshard([resD[c]["out"] for c in range(NCORE)])
        x = _host_ln(x, np.asarray(g2[layer]), np.asarray(be2[layer]))

    return x



# revision 8
# speedup vs baseline: 1.8813x; 1.8813x over previous
"""Deformable-Transformer encoder on 8 trn2 NeuronCores.

Sharding: data-parallel over batch x token-parallel within batch
(8 cores = 2 batches x 4 token-shards of 1360 tokens).

Three fused device programs (bass/Tile, SPMD on cores 0-7):
  P1: layer-0 value/offset/attn projections (bf16 matmuls)
  P2: layer-0 attn-out proj + residual + LN1 + FFN + residual + LN2,
      fused with layer-1 projections (val/off/attn of the new state)
  P3: layer-1 attn-out proj + ... + LN2 -> final output
The data-dependent bilinear sampling (sparse gather; this terminal's
runtime cannot load the GPSIMD gather ucode libraries) runs on host
between launches, as does the 16-way softmax feeding it.

Device-side techniques: bf16 matmuls (fp32 is 4 cyc/row on the PE),
residual adds via identity-matmul accumulation into PSUM, biases via
K=1 ones-row matmuls, LN stats via activation/TTR accum_out fused
with PSUM evacuation, LN apply via per-partition tensor_scalar,
PE-transposes between row-major (LN/residual) and col-major (matmul
lhsT) layouts, FFN hidden state resident in SBUF.
"""
import os
import sys
import types
import contextlib
import ctypes
import numpy as np

sys.path.insert(0, "/opt/trn_rl_repo")


def _install_ntff_hook():
    try:
        import antenv

        if hasattr(antenv, "axon_hooks"):
            return
        so_path = "/opt/axon/libaxon_pjrt.so"
        lib = ctypes.CDLL(so_path)
        if not hasattr(lib, "axon_start_nrt_profile"):
            hook = None
        else:
            lib.axon_start_nrt_profile.argtypes = [
                ctypes.POINTER(ctypes.c_int64), ctypes.c_size_t]
            lib.axon_start_nrt_profile.restype = ctypes.c_int64
            lib.axon_stop_nrt_profile.argtypes = [ctypes.c_char_p]
            lib.axon_stop_nrt_profile.restype = ctypes.c_int64

            @contextlib.contextmanager
            def hook(output_dir, device_ids):
                import jax
                jax.devices()
                if device_ids:
                    ids = (ctypes.c_int64 * len(device_ids))(*device_ids)
                    rc = lib.axon_start_nrt_profile(ids, len(device_ids))
                else:
                    rc = lib.axon_start_nrt_profile(None, 0)
                if rc != 0:
                    raise RuntimeError(f"start_nrt_profile rc={rc}")
                try:
                    yield
                finally:
                    lib.axon_stop_nrt_profile(str(output_dir).encode())

        m = types.ModuleType("antenv.axon_hooks")
        m.get_axon_ntff_profile_hook = lambda: hook
        m.set_axon_ntff_profile_hook = lambda h: None
        sys.modules["antenv.axon_hooks"] = m
        antenv.axon_hooks = m
    except Exception:
        pass


_install_ntff_hook()

import ml_dtypes  # noqa: E402
from concourse import bacc, tile, mybir, bass  # noqa: E402
from concourse.bass_utils import run_bass_kernel_spmd  # noqa: E402
from contextlib import ExitStack  # noqa: E402

F32 = mybir.dt.float32
BF16 = mybir.dt.bfloat16
BF_NP = ml_dtypes.bfloat16
AF = mybir.ActivationFunctionType
OP = mybir.AluOpType

SHAPES = ((64, 64), (32, 32), (16, 16), (8, 8))
LEVEL_STARTS = [0, 4096, 5120, 5376, 5440]
N_LEVELS, N_HEADS, N_POINTS = 4, 8, 4
D_MODEL, HEAD_DIM, D_FFN = 256, 32, 1024
LEN_IN, BATCH, NCORE = 5440, 2, 8
TPC = LEN_IN * BATCH // NCORE  # 1360 tokens per core
NT = 11                        # token tiles per core (10x128 + 80)
TPAD = NT * 128                # padded token rows for [tile,part] outputs
QT = [(i * 128, min(128, TPC - i * 128)) for i in range(NT)]
TOKSL = [(0, 512), (512, 512), (1024, TPC - 1024)]  # FFN1 token slices
EPS = 1e-5

HW_EXEC_NS = []  # per-launch exec times when BASS_TRACE=1
_PROGS = {}


def _nc():
    return bacc.Bacc("TRN2", target_bir_lowering=False, debug=False,
                     num_devices=NCORE)


def _ln_smalls(nc, sb, s1, s2, rstd, nmr, epsc, tag):
    """Batched LN small ops over [128, NT] stats.

    rstd = 1/sqrt(s2/256 - (s1/256)^2 + eps); nmr = -(s1/256)*rstd.
    """
    m2 = sb.tile([128, NT], F32, tag=tag + "m2")
    nc.vector.tensor_tensor(m2[:], s1[:], s1[:], op=OP.mult)
    v = sb.tile([128, NT], F32, tag=tag + "v")
    nc.vector.tensor_scalar(out=v[:], in0=m2[:], scalar1=-1.0 / (256.0 * 256.0),
                            scalar2=None, op0=OP.mult)
    t2 = sb.tile([128, NT], F32, tag=tag + "t2")
    nc.vector.tensor_scalar(out=t2[:], in0=s2[:], scalar1=1.0 / 256.0,
                            scalar2=None, op0=OP.mult)
    nc.vector.tensor_tensor(v[:], v[:], t2[:], op=OP.add)
    sd = sb.tile([128, NT], F32, tag=tag + "sd")
    nc.scalar.activation(sd[:], v[:], AF.Sqrt, bias=epsc[:, :1])
    nc.vector.reciprocal(rstd[:], sd[:])
    nc.vector.tensor_scalar(out=nmr[:], in0=s1[:], scalar1=-1.0 / 256.0,
                            scalar2=None, op0=OP.mult)
    nc.vector.tensor_tensor(nmr[:], nmr[:], rstd[:], op=OP.mult)


def _build_proj():
    """Layer-0 projections: val = x@Wv + bv, offaw = q@Woa + boa."""
    nc = _nc()
    xT_d = nc.dram_tensor("xT", [D_MODEL, TPC], BF16, kind="ExternalInput").ap()
    qT_d = nc.dram_tensor("qT", [D_MODEL, TPC], BF16, kind="ExternalInput").ap()
    wv_d = nc.dram_tensor("Wv", [D_MODEL, 256], BF16, kind="ExternalInput").ap()
    woa_d = nc.dram_tensor("Woa", [D_MODEL, 384], BF16,
                           kind="ExternalInput").ap()
    bvr_d = nc.dram_tensor("bv_row", [1, 256], BF16, kind="ExternalInput").ap()
    boar_d = nc.dram_tensor("boa_row", [1, 384], BF16,
                            kind="ExternalInput").ap()
    ones_d = nc.dram_tensor("ones_row", [1, 128], BF16,
                            kind="ExternalInput").ap()
    val_d = nc.dram_tensor("val", [TPAD, 256], BF16, kind="ExternalOutput").ap()
    oa_d = nc.dram_tensor("offaw", [TPAD, 384], BF16,
                          kind="ExternalOutput").ap()

    with tile.TileContext(nc) as tc, ExitStack() as ctx:
        sb = ctx.enter_context(tc.tile_pool(name="sb", bufs=1))
        psv = ctx.enter_context(tc.tile_pool(name="psv", bufs=2, space="PSUM"))
        pso = ctx.enter_context(tc.tile_pool(name="pso", bufs=2, space="PSUM"))

        xT = sb.tile([128, 2, TPC], BF16, tag="xT")
        nc.sync.dma_start(xT[:], xT_d.rearrange("(c p) n -> p c n", p=128))
        qT = sb.tile([128, 2, TPC], BF16, tag="qT")
        nc.sync.dma_start(qT[:], qT_d.rearrange("(c p) n -> p c n", p=128))
        wv = sb.tile([128, 2, 256], BF16, tag="wv")
        nc.sync.dma_start(wv[:], wv_d.rearrange("(c p) n -> p c n", p=128))
        woa = sb.tile([128, 2, 384], BF16, tag="woa")
        nc.sync.dma_start(woa[:], woa_d.rearrange("(c p) n -> p c n", p=128))
        bvr = sb.tile([1, 256], BF16, tag="bvr")
        nc.sync.dma_start(bvr[:], bvr_d[:])
        boar = sb.tile([1, 384], BF16, tag="boar")
        nc.sync.dma_start(boar[:], boar_d[:])
        ones = sb.tile([1, 128], BF16, tag="ones")
        nc.sync.dma_start(ones[:], ones_d[:])

        valb = sb.tile([128, NT, 256], BF16, tag="valb")
        oab = sb.tile([128, NT, 384], BF16, tag="oab")
        nc.gpsimd.memset(valb[:, NT - 1, :], 0.0)
        nc.gpsimd.memset(oab[:, NT - 1, :], 0.0)

        for t, (q0, sz) in enumerate(QT):
            pv = psv.tile([128, 256], F32, tag="pv")
            nc.tensor.matmul(pv[:sz], xT[:, 0, q0:q0 + sz], wv[:, 0, :],
                             start=True, stop=False)
            nc.tensor.matmul(pv[:sz], xT[:, 1, q0:q0 + sz], wv[:, 1, :],
                             start=False, stop=False)
            nc.tensor.matmul(pv[:sz], ones[:1, :sz], bvr[:1, :],
                             start=False, stop=True)
            nc.vector.tensor_copy(valb[:sz, t, :], pv[:sz])

            po = pso.tile([128, 384], F32, tag="po")
            nc.tensor.matmul(po[:sz], qT[:, 0, q0:q0 + sz], woa[:, 0, :],
                             start=True, stop=False)
            nc.tensor.matmul(po[:sz], qT[:, 1, q0:q0 + sz], woa[:, 1, :],
                             start=False, stop=False)
            nc.tensor.matmul(po[:sz], ones[:1, :sz], boar[:1, :],
                             start=False, stop=True)
            nc.scalar.copy(oab[:sz, t, :], po[:sz])

        nc.sync.dma_start(val_d.rearrange("(t p) n -> p t n", p=128), valb[:])
        nc.sync.dma_start(oa_d.rearrange("(t p) n -> p t n", p=128), oab[:])
    nc.compile()
    return nc


def _build_fused(has_next):
    """attn-out proj + residual + LN1 + FFN + residual + LN2 for one layer.

    has_next=True additionally computes the NEXT layer's projections from
    the new state and outputs (val, offaw, x1T). has_next=False outputs
    the final LN2 result transposed as fp32 (host transposes back).
    """
    nc = _nc()
    aT_d = nc.dram_tensor("attnT", [D_MODEL, TPC], BF16,
                          kind="ExternalInput").ap()
    xT_d = nc.dram_tensor("xT", [D_MODEL, TPC], BF16, kind="ExternalInput").ap()
    wo_d = nc.dram_tensor("Wo", [D_MODEL, 256], BF16, kind="ExternalInput").ap()
    bor_d = nc.dram_tensor("bo_row", [1, 256], BF16, kind="ExternalInput").ap()
    wl1_d = nc.dram_tensor("Wl1", [D_MODEL, D_FFN], BF16,
                           kind="ExternalInput").ap()
    bl1c_d = nc.dram_tensor("bl1_col", [128, 8], F32, kind="ExternalInput").ap()
    wl2_d = nc.dram_tensor("Wl2", [D_FFN, 256], BF16, kind="ExternalInput").ap()
    bl2r_d = nc.dram_tensor("bl2_row", [1, 256], BF16,
                            kind="ExternalInput").ap()
    g1c_d = nc.dram_tensor("g1_col", [128, 2], F32, kind="ExternalInput").ap()
    be1c_d = nc.dram_tensor("be1_col", [128, 2], F32, kind="ExternalInput").ap()
    g2c_d = nc.dram_tensor("g2_col", [128, 2], F32, kind="ExternalInput").ap()
    be2c_d = nc.dram_tensor("be2_col", [128, 2], F32, kind="ExternalInput").ap()
    i2_d = nc.dram_tensor("I2", [128, 2, 256], BF16, kind="ExternalInput").ap()
    ones_d = nc.dram_tensor("ones_row", [1, 128], BF16,
                            kind="ExternalInput").ap()
    epsc_d = nc.dram_tensor("eps_col", [128, 1], F32, kind="ExternalInput").ap()
    if has_next:
        posT_d = nc.dram_tensor("posT", [D_MODEL, TPC], BF16,
                                kind="ExternalInput").ap()
        wvn_d = nc.dram_tensor("Wvn", [D_MODEL, 256], BF16,
                               kind="ExternalInput").ap()
        woan_d = nc.dram_tensor("Woan", [D_MODEL, 384], BF16,
                                kind="ExternalInput").ap()
        bvnr_d = nc.dram_tensor("bvn_row", [1, 256], BF16,
                                kind="ExternalInput").ap()
        boanr_d = nc.dram_tensor("boan_row", [1, 384], BF16,
                                 kind="ExternalInput").ap()
        val_d = nc.dram_tensor("val", [TPAD, 256], BF16,
                               kind="ExternalOutput").ap()
        oa_d = nc.dram_tensor("offaw", [TPAD, 384], BF16,
                              kind="ExternalOutput").ap()
        x1T_d = nc.dram_tensor("x1T", [D_MODEL, TPC], BF16,
                               kind="ExternalOutput").ap()
    else:
        outT_d = nc.dram_tensor("outT", [D_MODEL, TPC], F32,
                                kind="ExternalOutput").ap()

    with tile.TileContext(nc) as tc, ExitStack() as ctx:
        sb = ctx.enter_context(tc.tile_pool(name="sb", bufs=1))
        xnp_ = ctx.enter_context(tc.tile_pool(name="xnp", bufs=3))
        sqp = ctx.enter_context(tc.tile_pool(name="sqp", bufs=2))
        psr = ctx.enter_context(tc.tile_pool(name="psr", bufs=2, space="PSUM"))
        psf = ctx.enter_context(tc.tile_pool(name="psf", bufs=2, space="PSUM"))
        pst = ctx.enter_context(tc.tile_pool(name="pst", bufs=2, space="PSUM"))
        if has_next:
            psa = ctx.enter_context(
                tc.tile_pool(name="psa", bufs=2, space="PSUM"))

        # ---- input DMAs (emission order ~ priority) ----
        aT = sb.tile([128, 2, TPC], BF16, tag="aT")
        nc.sync.dma_start(aT[:], aT_d.rearrange("(c p) n -> p c n", p=128))
        xT = sb.tile([128, 2, TPC], BF16, tag="xT")
        nc.sync.dma_start(xT[:], xT_d.rearrange("(c p) n -> p c n", p=128))
        wo = sb.tile([128, 2, 256], BF16, tag="wo")
        nc.sync.dma_start(wo[:], wo_d.rearrange("(c p) n -> p c n", p=128))
        i2 = sb.tile([128, 2, 256], BF16, tag="i2")
        nc.sync.dma_start(i2[:], i2_d[:])
        ones = sb.tile([1, 128], BF16, tag="ones")
        nc.sync.dma_start(ones[:], ones_d[:])
        bor = sb.tile([1, 256], BF16, tag="bor")
        nc.sync.dma_start(bor[:], bor_d[:])
        epsc = sb.tile([128, 1], F32, tag="epsc")
        nc.sync.dma_start(epsc[:], epsc_d[:])
        g1c = sb.tile([128, 2], F32, tag="g1c")
        nc.sync.dma_start(g1c[:], g1c_d[:])
        be1c = sb.tile([128, 2], F32, tag="be1c")
        nc.sync.dma_start(be1c[:], be1c_d[:])
        wl1 = sb.tile([128, 2, D_FFN], BF16, tag="wl1")
        nc.sync.dma_start(wl1[:], wl1_d.rearrange("(c p) n -> p c n", p=128))
        bl1c = sb.tile([128, 8], F32, tag="bl1c")
        nc.sync.dma_start(bl1c[:], bl1c_d[:])
        wl2 = sb.tile([128, 8, 256], BF16, tag="wl2")
        nc.sync.dma_start(wl2[:], wl2_d.rearrange("(c p) n -> p c n", p=128))
        bl2r = sb.tile([1, 256], BF16, tag="bl2r")
        nc.sync.dma_start(bl2r[:], bl2r_d[:])
        g2c = sb.tile([128, 2], F32, tag="g2c")
        nc.sync.dma_start(g2c[:], g2c_d[:])
        be2c = sb.tile([128, 2], F32, tag="be2c")
        nc.sync.dma_start(be2c[:], be2c_d[:])
        if has_next:
            posT = sb.tile([128, 2, TPC], BF16, tag="posT")
            nc.sync.dma_start(posT[:],
                              posT_d.rearrange("(c p) n -> p c n", p=128))
            wvn = sb.tile([128, 2, 256], BF16, tag="wvn")
            nc.sync.dma_start(wvn[:],
                              wvn_d.rearrange("(c p) n -> p c n", p=128))
            woan = sb.tile([128, 2, 384], BF16, tag="woan")
            nc.sync.dma_start(woan[:],
                              woan_d.rearrange("(c p) n -> p c n", p=128))
            bvnr = sb.tile([1, 256], BF16, tag="bvnr")
            nc.sync.dma_start(bvnr[:], bvnr_d[:])
            boanr = sb.tile([1, 384], BF16, tag="boanr")
            nc.sync.dma_start(boanr[:], boanr_d[:])

        # ---- persistent activation buffers ----
        r1b = sb.tile([128, NT, 256], BF16, tag="r1b")
        r2b = sb.tile([128, NT, 256], BF16, tag="r2b")
        x2T = sb.tile([128, 2, TPC], BF16, tag="x2T")
        hT = sb.tile([128, 8, TPC], BF16, tag="hT")
        s1a = sb.tile([128, NT], F32, tag="s1a")
        s2a = sb.tile([128, NT], F32, tag="s2a")
        s1b = sb.tile([128, NT], F32, tag="s1b")
        s2b = sb.tile([128, NT], F32, tag="s2b")
        nc.gpsimd.memset(s1a[:], 1.0)
        nc.gpsimd.memset(s2a[:], 1.0)
        nc.gpsimd.memset(s1b[:], 1.0)
        nc.gpsimd.memset(s2b[:], 1.0)
        rstd1 = sb.tile([128, NT], F32, tag="rstd1")
        nmr1 = sb.tile([128, NT], F32, tag="nmr1")
        rstd2 = sb.tile([128, NT], F32, tag="rstd2")
        nmr2 = sb.tile([128, NT], F32, tag="nmr2")
        if has_next:
            x1Tb = sb.tile([128, 2, TPC], BF16, tag="x1Tb")
            q1T = sb.tile([128, 2, TPC], BF16, tag="q1T")
            valb = sb.tile([128, NT, 256], BF16, tag="valb")
            oab = sb.tile([128, NT, 384], BF16, tag="oab")
            nc.gpsimd.memset(valb[:, NT - 1, :], 0.0)
            nc.gpsimd.memset(oab[:, NT - 1, :], 0.0)
        else:
            x1Tb = sb.tile([128, 2, TPC], F32, tag="x1Tb")

        # ---- P1: attn output proj + residual + bias; LN1 stats ----
        for t, (q0, sz) in enumerate(QT):
            pr = psr.tile([128, 256], F32, tag="pr")
            nc.tensor.matmul(pr[:sz], aT[:, 0, q0:q0 + sz], wo[:, 0, :],
                             start=True, stop=False)
            nc.tensor.matmul(pr[:sz], aT[:, 1, q0:q0 + sz], wo[:, 1, :],
                             start=False, stop=False)
            nc.tensor.matmul(pr[:sz], xT[:, 0, q0:q0 + sz], i2[:, 0, :],
                             start=False, stop=False)
            nc.tensor.matmul(pr[:sz], xT[:, 1, q0:q0 + sz], i2[:, 1, :],
                             start=False, stop=False)
            nc.tensor.matmul(pr[:sz], ones[:1, :sz], bor[:1, :],
                             start=False, stop=True)
            nc.scalar.activation(r1b[:sz, t, :], pr[:sz], AF.Identity,
                                 accum_out=s1a[:sz, t:t + 1])
            sq = sqp.tile([128, 256], BF16, tag="sq")
            nc.scalar.activation(sq[:sz], r1b[:sz, t, :], AF.Square,
                                 accum_out=s2a[:sz, t:t + 1])

        # ---- P2: LN1 smalls ----
        _ln_smalls(nc, sb, s1a, s2a, rstd1, nmr1, epsc, "ln1")

        # ---- P3: LN1 apply + transpose -> x2T (g1, be1 in evac) ----
        for t, (q0, sz) in enumerate(QT):
            xm = xnp_.tile([128, 256], BF16, tag="xm")
            nc.vector.tensor_scalar(
                out=xm[:sz], in0=r1b[:sz, t, :], scalar1=rstd1[:sz, t:t + 1],
                scalar2=None, op0=OP.mult)
            xn = xnp_.tile([128, 256], BF16, tag="xn")
            nc.vector.tensor_scalar(
                out=xn[:sz], in0=xm[:sz], scalar1=nmr1[:sz, t:t + 1],
                scalar2=None, op0=OP.add)
            pt = pst.tile([128, 2, 128], BF16, tag="pt")
            for c in range(2):
                nc.tensor.transpose(pt[:, c, :sz],
                                    xn[:sz, c * 128:(c + 1) * 128],
                                    i2[:sz, 0, :sz])
                nc.scalar.activation(x2T[:, c, q0:q0 + sz], pt[:, c, :sz],
                                     AF.Identity, scale=g1c[:, c:c + 1],
                                     bias=be1c[:, c:c + 1])

        # ---- P4: FFN1 (stationary Wl1 slices) -> hT with ReLU+bias ----
        for si, (t0, tn) in enumerate(TOKSL):
            for n in range(8):
                pf = psf.tile([128, 512], F32, tag="pf")
                nc.tensor.matmul(pf[:, :tn], wl1[:, 0, n * 128:(n + 1) * 128],
                                 x2T[:, 0, t0:t0 + tn], start=True, stop=False)
                nc.tensor.matmul(pf[:, :tn], wl1[:, 1, n * 128:(n + 1) * 128],
                                 x2T[:, 1, t0:t0 + tn], start=False, stop=True)
                if n % 2 == 0:
                    nc.scalar.activation(hT[:, n, t0:t0 + tn], pf[:, :tn],
                                         AF.Relu, bias=bl1c[:, n:n + 1])
                else:
                    nc.vector.tensor_scalar(
                        out=hT[:, n, t0:t0 + tn], in0=pf[:, :tn],
                        scalar1=bl1c[:, n:n + 1], scalar2=0.0,
                        op0=OP.add, op1=OP.max)

        # ---- P5: FFN2 + x2 residual + bias; LN2 stats ----
        for t, (q0, sz) in enumerate(QT):
            p2 = psr.tile([128, 256], F32, tag="pr")
            for ch in range(8):
                nc.tensor.matmul(p2[:sz], hT[:, ch, q0:q0 + sz],
                                 wl2[:, ch, :], start=(ch == 0), stop=False)
            nc.tensor.matmul(p2[:sz], x2T[:, 0, q0:q0 + sz], i2[:, 0, :],
                             start=False, stop=False)
            nc.tensor.matmul(p2[:sz], x2T[:, 1, q0:q0 + sz], i2[:, 1, :],
                             start=False, stop=False)
            nc.tensor.matmul(p2[:sz], ones[:1, :sz], bl2r[:1, :],
                             start=False, stop=True)
            nc.scalar.activation(r2b[:sz, t, :], p2[:sz], AF.Identity,
                                 accum_out=s1b[:sz, t:t + 1])
            sq = sqp.tile([128, 256], BF16, tag="sq")
            nc.scalar.activation(sq[:sz], r2b[:sz, t, :], AF.Square,
                                 accum_out=s2b[:sz, t:t + 1])

        # ---- P6: LN2 smalls ----
        _ln_smalls(nc, sb, s1b, s2b, rstd2, nmr2, epsc, "ln2")

        # ---- P7: LN2 apply + transpose -> x1Tb (g2, be2 in evac) ----
        for t, (q0, sz) in enumerate(QT):
            xm = xnp_.tile([128, 256], BF16, tag="xm")
            nc.vector.tensor_scalar(
                out=xm[:sz], in0=r2b[:sz, t, :], scalar1=rstd2[:sz, t:t + 1],
                scalar2=None, op0=OP.mult)
            xn = xnp_.tile([128, 256], BF16, tag="xn")
            nc.vector.tensor_scalar(
                out=xn[:sz], in0=xm[:sz], scalar1=nmr2[:sz, t:t + 1],
                scalar2=None, op0=OP.add)
            pt = pst.tile([128, 2, 128], BF16, tag="pt")
            for c in range(2):
                nc.tensor.transpose(pt[:, c, :sz],
                                    xn[:sz, c * 128:(c + 1) * 128],
                                    i2[:sz, 0, :sz])
                nc.scalar.activation(x1Tb[:, c, q0:q0 + sz], pt[:, c, :sz],
                                     AF.Identity, scale=g2c[:, c:c + 1],
                                     bias=be2c[:, c:c + 1])

        if has_next:
            # ---- P8: q1T = x1T + posT ----
            for c in range(2):
                for (t0, tn) in TOKSL:
                    nc.vector.tensor_tensor(q1T[:, c, t0:t0 + tn],
                                            x1Tb[:, c, t0:t0 + tn],
                                            posT[:, c, t0:t0 + tn], op=OP.add)
            # ---- P9: next-layer projections ----
            for t, (q0, sz) in enumerate(QT):
                pv = psr.tile([128, 256], F32, tag="pr")
                nc.tensor.matmul(pv[:sz], x1Tb[:, 0, q0:q0 + sz],
                                 wvn[:, 0, :], start=True, stop=False)
                nc.tensor.matmul(pv[:sz], x1Tb[:, 1, q0:q0 + sz],
                                 wvn[:, 1, :], start=False, stop=False)
                nc.tensor.matmul(pv[:sz], ones[:1, :sz], bvnr[:1, :],
                                 start=False, stop=True)
                nc.vector.tensor_copy(valb[:sz, t, :], pv[:sz])

                po = psa.tile([128, 384], F32, tag="po")
                nc.tensor.matmul(po[:sz], q1T[:, 0, q0:q0 + sz], woan[:, 0, :],
                                 start=True, stop=False)
                nc.tensor.matmul(po[:sz], q1T[:, 1, q0:q0 + sz], woan[:, 1, :],
                                 start=False, stop=False)
                nc.tensor.matmul(po[:sz], ones[:1, :sz], boanr[:1, :],
                                 start=False, stop=True)
                nc.scalar.copy(oab[:sz, t, :], po[:sz])

            nc.sync.dma_start(val_d.rearrange("(t p) n -> p t n", p=128),
                              valb[:])
            nc.sync.dma_start(oa_d.rearrange("(t p) n -> p t n", p=128),
                              oab[:])
            nc.sync.dma_start(x1T_d.rearrange("(c p) n -> p c n", p=128),
                              x1Tb[:])
        else:
            nc.sync.dma_start(outT_d.rearrange("(c p) n -> p c n", p=128),
                              x1Tb[:])
    nc.compile()
    return nc


def _run(prog, in_maps):
    trace = bool(os.environ.get("BASS_TRACE"))
    res = run_bass_kernel_spmd(prog, in_maps, core_ids=list(range(NCORE)),
                               trace=trace)
    if res.exec_time_ns:
        HW_EXEC_NS.append(res.exec_time_ns)
    return res.results


def _bf(a):
    return np.ascontiguousarray(np.asarray(a, np.float32)).astype(BF_NP)


def _ref_points(valid_ratios):
    refs = []
    for lvl, (H, W) in enumerate(SHAPES):
        gy, gx = np.meshgrid(np.arange(H, dtype=np.float32) + 0.5,
                             np.arange(W, dtype=np.float32) + 0.5,
                             indexing="ij")
        ry = gy.reshape(-1)[None] / (valid_ratios[:, lvl, 1][:, None] * H)
        rx = gx.reshape(-1)[None] / (valid_ratios[:, lvl, 0][:, None] * W)
        refs.append(np.stack([rx, ry], -1))
    ref = np.concatenate(refs, 1)
    return ref[:, :, None, :] * valid_ratios[:, None]


def _host_sample(value, off, aw, ref_pts):
    """value[N,L,8,32] off[N,L,256] aw[N,L,128](softmaxed) -> [N,L,256]"""
    N, Lq = off.shape[:2]
    off = off.reshape(N, Lq, N_HEADS, N_LEVELS, N_POINTS, 2)
    aw = aw.reshape(N, Lq, N_HEADS, N_LEVELS, N_POINTS)
    normalizer = np.array([[w, h] for h, w in SHAPES], np.float32)
    loc = (ref_pts[:, :, None, :, None, :]
           + off / normalizer[None, None, None, :, None, :])
    acc = np.zeros((N, N_HEADS, Lq, HEAD_DIM), np.float32)
    for lvl, (H, W) in enumerate(SHAPES):
        s = LEVEL_STARTS[lvl]
        val = value[:, s:s + H * W].transpose(0, 2, 1, 3)
        x = loc[:, :, :, lvl, :, 0] * W - 0.5
        y = loc[:, :, :, lvl, :, 1] * H - 0.5
        x0 = np.floor(x)
        y0 = np.floor(y)
        wx1 = x - x0
        wy1 = y - y0
        ix0 = x0.astype(np.int64)
        iy0 = y0.astype(np.int64)

        def corner(ix, iy, w):
            valid = (ix >= 0) & (ix < W) & (iy >= 0) & (iy < H)
            idx = np.clip(iy, 0, H - 1) * W + np.clip(ix, 0, W - 1)
            idx = idx.transpose(0, 2, 1, 3).reshape(N, N_HEADS, Lq * N_POINTS)
            g = np.take_along_axis(val, idx[..., None], axis=2)
            g = g.reshape(N, N_HEADS, Lq, N_POINTS, HEAD_DIM)
            w = np.where(valid, w, 0.0).transpose(0, 2, 1, 3)
            return g * w[..., None].astype(np.float32)

        sampled = (corner(ix0, iy0, (1 - wx1) * (1 - wy1))
                   + corner(ix0 + 1, iy0, wx1 * (1 - wy1))
                   + corner(ix0, iy0 + 1, (1 - wx1) * wy1)
                   + corner(ix0 + 1, iy0 + 1, wx1 * wy1))
        acc += (sampled * aw[:, :, :, lvl].transpose(0, 2, 1, 3)[..., None]
                ).sum(3)
    return acc.transpose(0, 2, 1, 3).reshape(N, Lq, D_MODEL)


def _softmax_gather(value, offaw, ref_pts):
    """value [N,L,256] f32, offaw [N,L,384] f32 -> attn [N,L,256] f32."""
    aw = offaw[:, :, 256:].reshape(BATCH, LEN_IN, N_HEADS, 16)
    aw = aw - aw.max(-1, keepdims=True)
    e = np.exp(aw)
    aw = (e / e.sum(-1, keepdims=True)).reshape(BATCH, LEN_IN, 128)
    return _host_sample(value.reshape(BATCH, LEN_IN, N_HEADS, HEAD_DIM),
                        offaw[:, :, :256], aw, ref_pts)


def kernel(src, pos, valid_ratios, Wv, bv, Woff, boff, Wa, ba, Wo, bo,
           g1, be1, Wl1, bl1, Wl2, bl2, g2, be2):
    src = np.asarray(src, np.float32)
    pos = np.asarray(pos, np.float32)
    valid_ratios = np.asarray(valid_ratios, np.float32)
    HW_EXEC_NS.clear()

    if "P" not in _PROGS:
        _PROGS["P"] = _build_proj()
        _PROGS["F2"] = _build_fused(True)
        _PROGS["F3"] = _build_fused(False)

    ref_pts = _ref_points(valid_ratios)

    def shard(full):  # [2,5440,F] -> list of 8 [TPC,F]
        return [np.ascontiguousarray(full[c // 4, (c % 4) * TPC:
                                          (c % 4 + 1) * TPC])
                for c in range(NCORE)]

    def unshard(parts):  # list of 8 [TPC,F] -> [2,5440,F]
        F = parts[0].shape[-1]
        out = np.empty((BATCH, LEN_IN, F), np.float32)
        for c in range(NCORE):
            out[c // 4, (c % 4) * TPC:(c % 4 + 1) * TPC] = \
                np.asarray(parts[c], np.float32)
        return out

    # host-side constant prep
    I256 = np.eye(256, dtype=np.float32)
    I2 = np.ascontiguousarray(
        I256.reshape(2, 128, 256).transpose(1, 0, 2)).astype(BF_NP)
    ones_row = np.ones((1, 128), BF_NP)
    epsc = np.full((128, 1), EPS, np.float32)

    def col128(v, ncol):  # [ncol*128] -> [128, ncol] f32
        return np.ascontiguousarray(
            np.asarray(v, np.float32).reshape(ncol, 128).T)

    Woa = [np.concatenate([np.asarray(Woff[l]), np.asarray(Wa[l])], axis=1)
           for l in range(2)]
    boa = [np.concatenate([np.asarray(boff[l]), np.asarray(ba[l])])
           for l in range(2)]

    xs = shard(src)
    qs = shard(src + pos)
    poss = shard(pos)

    # ---- launch 1: layer-0 projections ----
    in_maps = [{
        "xT": _bf(xs[c].T),
        "qT": _bf(qs[c].T),
        "Wv": _bf(Wv[0]),
        "Woa": _bf(Woa[0]),
        "bv_row": _bf(np.asarray(bv[0])[None, :]),
        "boa_row": _bf(boa[0][None, :]),
        "ones_row": ones_row,
    } for c in range(NCORE)]
    res = _run(_PROGS["P"], in_maps)
    value0 = unshard([res[c]["val"][:TPC] for c in range(NCORE)])
    offaw0 = unshard([res[c]["offaw"][:TPC] for c in range(NCORE)])

    attn0 = _softmax_gather(value0, offaw0, ref_pts)
    ats = shard(attn0)

    # ---- launch 2: layer-0 body + layer-1 projections ----
    in_maps = [{
        "attnT": _bf(ats[c].T),
        "xT": _bf(xs[c].T),
        "Wo": _bf(Wo[0]),
        "bo_row": _bf(np.asarray(bo[0])[None, :]),
        "Wl1": _bf(Wl1[0]),
        "bl1_col": col128(bl1[0], 8),
        "Wl2": _bf(Wl2[0]),
        "bl2_row": _bf(np.asarray(bl2[0])[None, :]),
        "g1_col": col128(g1[0], 2),
        "be1_col": col128(be1[0], 2),
        "g2_col": col128(g2[0], 2),
        "be2_col": col128(be2[0], 2),
        "I2": I2,
        "ones_row": ones_row,
        "eps_col": epsc,
        "posT": _bf(poss[c].T),
        "Wvn": _bf(Wv[1]),
        "Woan": _bf(Woa[1]),
        "bvn_row": _bf(np.asarray(bv[1])[None, :]),
        "boan_row": _bf(boa[1][None, :]),
    } for c in range(NCORE)]
    res = _run(_PROGS["F2"], in_maps)
    value1 = unshard([res[c]["val"][:TPC] for c in range(NCORE)])
    offaw1 = unshard([res[c]["offaw"][:TPC] for c in range(NCORE)])
    x1Ts = [res[c]["x1T"] for c in range(NCORE)]

    attn1 = _softmax_gather(value1, offaw1, ref_pts)
    ats1 = shard(attn1)

    # ---- launch 3: layer-1 body -> final output ----
    in_maps = [{
        "attnT": _bf(ats1[c].T),
        "xT": np.ascontiguousarray(x1Ts[c]),
        "Wo": _bf(Wo[1]),
        "bo_row": _bf(np.asarray(bo[1])[None, :]),
        "Wl1": _bf(Wl1[1]),
        "bl1_col": col128(bl1[1], 8),
        "Wl2": _bf(Wl2[1]),
        "bl2_row": _bf(np.asarray(bl2[1])[None, :]),
        "g1_col": col128(g1[1], 2),
        "be1_col": col128(be1[1], 2),
        "g2_col": col128(g2[1], 2),
        "be2_col": col128(be2[1], 2),
        "I2": I2,
        "ones_row": ones_row,
        "eps_col": epsc,
    } for c in range(NCORE)]
    res = _run(_PROGS["F3"], in_maps)
    out = unshard([np.asarray(res[c]["outT"], np.float32).T
                   for c in range(NCORE)])
    return out
